# revision 1
# baseline (speedup 1.0000x reference)
"""Trainium2 Bass kernel for nn_CoLL_78065325572576 (moe_routing).

Reference computation (per voxel v of x[B,H,W,C], nb=8 bins):
    b_v   = floor(8*x_v)                       (bin index)
    temp  = co[i, b_v] * x_v                   (8 channels)
    conv  = depthwise 3x3x3 conv over (H,W,C)  (SAME pad, 8 channels)
    out_v = conv[v, b_v] + bias[b_v]

Kernel formulation used here (all equalities exact):
    s_q[v]  = x_v * 1[b_v == q]                 (mask-routed fields)
    out_v   = sum_p 1[b_v==p] * ( sum_{dv,q} K[dv,p]*co[p,q] * s_q[v+dv] + bias[p] )

Device mapping (per core, pure data-parallel over 8 cores = batch x W-half):
  - x replicated by DMA into partitions (q=8, hs=16); h tiled in 10
    overlapping windows of 16 rows (stride 14, valid interior 14).
  - ROUTE (custom DVE op): s = x masked per bin-group, bf16.
  - CONV: 9 accumulating banded bf16 matmuls on TensorE; lhsT
    [(q,hs),(p,hs')] = K[hs-hs'+1, dw+1, dc+1, p]*co[p,q] folds the
    8x8 channel mix and the dh taps; (dw,dc) are free-dim AP shifts.
  - SELECT (custom DVE op): masked = (x in bin p) ? g + bias[p] : 0.
  - REDUCE: ones-banded float32r matmul sums the 8 p-groups.
  - ScalarE drains PSUM; DMA writes the output shard.
"""

import numpy as np
import ml_dtypes

NB = 8
B, H, W, C = 4, 128, 128, 64
WS = 64            # output W per core
WH = WS + 2        # input W incl. halo
WIN = 16           # h-window rows (one partition group)
VALID = 14         # valid output rows per window
NWIN = 10          # h windows (stride 14): covers h in [0,128)
NCORES = 8
NSTRIPES = 4
WSTR = WS // NSTRIPES        # 16 output w per stripe
WSTR_IN = WSTR + 2           # 18 input w per stripe
FSTR_IN = WSTR_IN * C        # 1152
CP = C + 2                   # c padded with zero border cols in routed tensor
FSTR_SL = WSTR_IN * CP       # 1188
TAPS = [(dw, dc) for dc in (0, -1, 1) for dw in (-1, 0, 1)]  # dc=0 first

_prog_cache: dict = {}


# --------------------------------------------------------------------------- #
# custom DVE ops (registered at import into concourse.dve_ops)                #
# --------------------------------------------------------------------------- #

def _register_ops():
    from concourse import dve_ops
    from concourse.dve_spec import (
        Spec, Src0, Src1, C0, C1, C2, Zero, lower, select, _has_src1,
    )
    from concourse.dve_uop import DveOpSpec

    if "ANT_ROUTE_BIN8" in dve_ops._SUB_OPCODE_FOR_NAME:
        ops = {op.name: op for op in dve_ops.OPS}
        return ops["ANT_ROUTE_BIN8"], ops["ANT_SEL_BIN8"]

    def reg(name, spec, subdim=False):
        row = dve_ops._CUSTOM_DVE_ROW_BASE + len(dve_ops.OPS)
        assert row < 0x20, "custom DVE opcode rows exhausted"
        dve_ops._SUB_OPCODE_FOR_NAME[name] = row
        shas = {}
        for ver in ("v3", "v4"):
            try:
                s = DveOpSpec(name=name, opcode=row,
                              uops=lower(spec, ver=ver),
                              rd1_en=_has_src1(spec))
                shas[ver] = s.sha(ver)
            except Exception:
                pass
        op = dve_ops.DveOp(name, spec, subdim=subdim, uops_sha=shas)
        dve_ops.OPS.append(op)
        dve_ops.CUSTOM_DVE_SPECS[name] = spec
        return op

    def _bc(v):
        return v if isinstance(v, float) else np.asarray(v).reshape(-1, 1)

    # s = x if (x >= lo) & (x < hi) else 0   (lo/hi per-partition scalars)
    route = reg("ANT_ROUTE_BIN8", Spec(
        body=select((Src0 >= C0) & (Src0 < C1), Src0, Zero),
        reference=lambda in0, in1, s0, s1, imm2: np.where(
            (in0 >= _bc(s0)) & (in0 < _bc(s1)), in0, 0.0).astype(np.float32),
    ))

    # masked = (x >= lo) & (x < lo + width) ? g + bias : 0
    #   in0 = g (PSUM), in1 = x (center voxel), s0 = lo, s1 = bias,
    #   imm2 = 1/8 bin width (compile-time literal)
    selb = reg("ANT_SEL_BIN8", Spec(
        body=select((Src1 >= C0) & (Src1 < (C0 + C2)), Src0 + C1, Zero),
        reference=lambda in0, in1, s0, s1, imm2: np.where(
            (in1 >= _bc(s0)) & (in1 < (_bc(s0) + imm2)),
            in0 + _bc(s1), 0.0).astype(np.float32),
    ))
    return route, selb


# --------------------------------------------------------------------------- #
# host-side constant construction                                             #
# --------------------------------------------------------------------------- #

def _band_lhsT(dw_kernel, co_matrix, dwi, dci):
    """lhsT[(q,hs),(p,hs')] = K[dh+1, dwi+1, dci+1, p] * co[p,q], dh=hs-hs',
    for hs' in [1,15), |dh| <= 1."""
    K = np.asarray(dw_kernel, np.float32)       # [3,3,3,1,8]
    co = np.asarray(co_matrix, np.float32)      # [8,8]
    lhsT = np.zeros((128, 128), np.float32)
    hsp = np.arange(1, 15)
    for q in range(NB):
        for p in range(NB):
            for dh in (-1, 0, 1):
                a = K[dh + 1, dwi + 1, dci + 1, 0, p] * co[p, q]
                lhsT[q * WIN + hsp + dh, p * WIN + hsp] = a
    return lhsT


def _make_consts(co_matrix, dw_kernel, dw_bias):
    conv_w = np.stack([_band_lhsT(dw_kernel, co_matrix, dwi, dci)
                       for (dwi, dci) in TAPS])          # [9,128,128]
    red_w = np.zeros((128, VALID), np.float32)
    for p in range(NB):
        for hspp in range(VALID):
            red_w[p * WIN + hspp + 1, hspp] = 1.0
    part = np.arange(128)
    bins_lo = ((part // WIN) / NB).astype(np.float32).reshape(128, 1)
    bias_p = np.asarray(dw_bias, np.float32)[part // WIN].reshape(128, 1)
    return {
        "conv_w": conv_w.astype(ml_dtypes.bfloat16),
        "red_w": red_w,
        "bins_lo": bins_lo,
        "bias_p": bias_p,
    }


def _shard(x, core):
    b, wh = core // 2, core % 2
    xp = np.zeros((H, WH, C), np.float32)
    lo, hi = wh * WS - 1, wh * WS + WS + 1
    slo, shi = max(0, lo), min(W, hi)
    xp[:, slo - lo:shi - lo, :] = x[b, :, slo:shi, :]
    return xp


# --------------------------------------------------------------------------- #
# device program                                                              #
# --------------------------------------------------------------------------- #

def _build_program(reps=1, with_bias=False):
    import concourse.mybir as mybir
    import concourse.tile as tile
    from concourse import bacc
    import bass_rust

    def raw_ap(base_ap, dims, offset):
        a = base_ap.copy()
        a.ap = bass_rust.VecI64Pair(dims)
        a.offset = offset
        return a

    ROUTE, SELB = _register_ops()
    f32 = mybir.dt.float32
    bf16 = mybir.dt.bfloat16
    f32r = mybir.dt.float32r

    nc = bacc.Bacc("TRN2", target_bir_lowering=False, debug=False)
    x_d = nc.dram_tensor("x_s", [H, WH, C], f32, kind="ExternalInput")
    cw_d = nc.dram_tensor("conv_w", [9, 128, 128], bf16, kind="ExternalInput")
    rw_d = nc.dram_tensor("red_w", [128, VALID], f32r, kind="ExternalInput")
    lo_d = nc.dram_tensor("bins_lo", [128, 1], f32, kind="ExternalInput")
    bi_d = nc.dram_tensor("bias_p", [128, 1], f32, kind="ExternalInput")
    out_d = nc.dram_tensor("out_s", [H, WS, C], f32, kind="ExternalOutput")

    with tile.TileContext(nc) as tc:
        with (
            tc.tile_pool(name="const", bufs=1) as cpool,
            tc.tile_pool(name="xr", bufs=2) as xrpool,
            tc.tile_pool(name="sl", bufs=2) as slpool,
            tc.tile_pool(name="mk", bufs=2) as mkpool,
            tc.tile_pool(name="msk", bufs=2) as mskpool,
            tc.tile_pool(name="ost", bufs=3) as ostpool,
            tc.tile_pool(name="ps", bufs=3, space="PSUM") as pspool,
            tc.tile_pool(name="ps2", bufs=3, space="PSUM") as ps2pool,
        ):
            cw = cpool.tile([128, 9 * 128], bf16)
            nc.sync.dma_start(
                cw[:, :],
                raw_ap(cw_d[0], [[128, 128], [128 * 128, 9], [1, 128]], 0))
            rw = cpool.tile([128, VALID], f32r)
            nc.sync.dma_start(rw[:], rw_d[:])
            lo = cpool.tile([128, 1], f32)
            nc.sync.dma_start(lo[:], lo_d[:])
            bi = cpool.tile([128, 1], f32)
            nc.sync.dma_start(bi[:], bi_d[:])
            hi = cpool.tile([128, 1], f32)
            nc.vector.tensor_scalar_add(hi[:], lo[:], 1.0 / NB)

            for rep in range(reps):
              for st in range(NSTRIPES):
                  # ---- load x replicated into (q, hs) x (hw, w, c) ---------- #
                  xr0 = xrpool.tile([128, 1, FSTR_IN], f32, tag="xr0")
                  xr = xrpool.tile([128, NWIN, FSTR_IN], f32, tag="xr")
                  # zero rows whose h falls outside [0, 128)
                  nc.gpsimd.memset(xr0[:, 0, :], 0.0)
                  nc.gpsimd.memset(xr[:, 9, :], 0.0)
                  wb = st * WSTR
                  for q in range(NB):
                      nc.sync.dma_start(
                          xr0[q * WIN + 1:(q + 1) * WIN, 0, :],
                          x_d[0:15, wb:wb + WSTR_IN, :])
                  for q in range(NB):
                      nc.sync.dma_start(
                          xr[q * WIN:(q + 1) * WIN, 1:9, :],
                          raw_ap(x_d[0:WIN, 0:WSTR_IN, :],
                                 [[WH * C, WIN], [14 * WH * C, 8],
                                  [1, FSTR_IN]],
                                 13 * WH * C + wb * C))
                      nc.sync.dma_start(
                          xr[q * WIN:q * WIN + 3, 9, :],
                          x_d[125:128, wb:wb + WSTR_IN, :])

                  # ---- route to bf16 bin fields (c padded to 66) ------------ #
                  sl = slpool.tile([128, NWIN, WSTR_IN, CP], bf16, tag="sl")
                  # zero the c-border columns once per stripe
                  nc.gpsimd.memset(sl[:, :, :, 0], 0.0)
                  nc.gpsimd.memset(sl[:, :, :, CP - 1], 0.0)
                  for hw in range(NWIN):
                      xsrc = xr0[:, 0, :] if hw == 0 else xr[:, hw, :]
                      nc.vector._custom_dve(
                          ROUTE, out=sl[:, hw, :, 1:1 + C],
                          in0=xsrc.rearrange("p (w c) -> p w c", c=C),
                          s0=lo[:], s1=hi[:])

                  # ---- conv + select + reduce per 512-col chunk ------------- #
                  # Tail stages (select/reduce/drain) are emitted one chunk
                  # behind the conv matmuls so PE never waits on the DVE
                  # select of the chunk it just accumulated.
                  pend = []

                  def flush_tail(pend=pend, st=st):
                      if not pend:
                          return
                      ps, hw, wc = pend.pop(0)
                      mk = mkpool.tile([128, 512], f32r, tag="mk",
                                       name=f"mk_{st}_{hw}_{wc}")
                      xcsrc = xr0 if hw == 0 else xr
                      xcen = xcsrc[:, hw if hw else 0,
                                   (wc * 8 + 1) * C:(wc * 8 + 9) * C]
                      nc.vector._custom_dve(
                          SELB, out=mk[:], in0=ps[:],
                          in1=xcen, s0=lo[:], s1=bi[:], imm2=1.0 / NB)

                      p2 = ps2pool.tile([VALID, 512], f32, tag="p2",
                                        name=f"p2_{st}_{hw}_{wc}")
                      nc.tensor.matmul(p2[:], rw[:], mk[:],
                                       start=True, stop=True)

                      ost = osts[hw]
                      nc.scalar.copy(ost[:, wc * 512:(wc + 1) * 512], p2[:])
                      if wc == WSTR // 8 - 1:
                          rows = VALID if hw < 9 else 2
                          dst = out_d[14 * hw:14 * hw + rows,
                                      st * WSTR:(st + 1) * WSTR, :]
                          nc.gpsimd.dma_start(dst, ost[0:rows, :])

                  osts = {}
                  for hw in range(NWIN):
                      osts[hw] = ostpool.tile([VALID, WSTR * C], f32,
                                              tag="ost",
                                              name=f"ost_{st}_{hw}")
                      for wc in range(WSTR // 8):
                          ps = pspool.tile([128, 512], f32, tag="ps",
                                           name=f"ps_{st}_{hw}_{wc}")
                          for ti, (dwi, dci) in enumerate(TAPS):
                              w0 = wc * 8 + 1 + dwi
                              rhs = sl[:, hw, w0:w0 + 8, 1 + dci:1 + dci + C]
                              nc.tensor.matmul(
                                  ps[:], cw[:, ti * 128:(ti + 1) * 128], rhs,
                                  start=(ti == 0), stop=(ti == len(TAPS) - 1))
                          pend.append((ps, hw, wc))
                          if len(pend) > 1:
                              flush_tail()
                  while pend:
                      flush_tail()

    nc.compile()
    return nc


def _get_program(reps=1, with_bias=False):
    key = (reps, with_bias)
    if key not in _prog_cache:
        _prog_cache[key] = _build_program(reps, with_bias)
    return _prog_cache[key]


# --------------------------------------------------------------------------- #
# entry point                                                                 #
# --------------------------------------------------------------------------- #

def kernel(x, co_matrix, dw_kernel, dw_bias):
    from concourse.bass_utils import run_bass_kernel_spmd

    x = np.asarray(x, np.float32)
    consts = _make_consts(co_matrix, dw_kernel, dw_bias)
    nc = _get_program(with_bias=bool(np.any(np.asarray(dw_bias))))

    in_maps = []
    for core in range(NCORES):
        m = {"x_s": _shard(x, core)}
        m.update(consts)
        in_maps.append(m)

    res = run_bass_kernel_spmd(nc, in_maps, core_ids=list(range(NCORES)))
    out = np.zeros((B, H, W, C), np.float32)
    for core in range(NCORES):
        b, wh = core // 2, core % 2
        out[b, :, wh * WS:(wh + 1) * WS, :] = res.results[core]["out_s"]
    return out



# revision 2
# speedup vs baseline: 3000.0456x; 3000.0456x over previous
"""Trainium2 Bass kernel for nn_CoLL_78065325572576 (moe_routing).

Reference computation (per voxel v of x[B,H,W,C], nb=8 bins):
    b_v   = floor(8*x_v)                       (bin index)
    temp  = co[i, b_v] * x_v                   (8 channels)
    conv  = depthwise 3x3x3 conv over (H,W,C)  (SAME pad, 8 channels)
    out_v = conv[v, b_v] + bias[b_v]

Kernel formulation (all equalities exact):
    s_q[v]  = x_v * 1[b_v == q]                 (mask-routed fields)
    out_v   = sum_p 1[b_v==p] * ( sum_{dv,q} K[dv,p]*co[p,q] * s_q[v+dv] + bias[p] )

Device mapping (per core, pure data-parallel over 8 cores = batch x W-half):
  - per h-window (16 rows, stride 14): ONE partition-replicating DMA loads
    x rows into all 8 bin-groups (src AP [[0,8],[row,16],[1,F]]).
  - ROUTE (custom DVE op): s = x masked per bin-group, bf16.
  - CONV: 9 accumulating banded bf16 matmuls on TensorE; lhsT
    [(q,hs),(p,hs')] = K[hs-hs'+1, dw+1, dc+1, p]*co[p,q] folds the
    8x8 channel mix and the dh taps; (dw,dc) are free-dim AP shifts.
  - SELECT (custom DVE op): masked = (x in bin p) ? g + bias[p] : 0 (bf16).
  - REDUCE: per-window banded bf16 matmul scatters the 14 valid rows into a
    stripe-wide PSUM accumulator [128, 1024] (10 windows accumulate).
  - One ScalarE drain + one output DMA per stripe.
"""

import numpy as np
import ml_dtypes

NB = 8
B, H, W, C = 4, 128, 128, 64
WS = 64            # output W per core
WH = WS + 2        # input W incl. halo
WIN = 16           # h-window rows (one partition group)
VALID = 14         # valid output rows per window
NWIN = 10          # h windows (stride 14): covers h in [0,128)
NCORES = 8
NSTRIPES = 4
WSTR = WS // NSTRIPES        # 16 output w per stripe
WSTR_IN = WSTR + 2           # 18 input w per stripe
FSTR_IN = WSTR_IN * C        # 1152
CP = C + 2                   # c padded with zero border cols in routed tensor
TAPS = [(dw, dc) for dc in (0, -1, 1) for dw in (-1, 0, 1)]  # dc=0 first
PP = WH * C                  # x_s row pitch (elements)
PH = 14 * (NWIN - 1) + WIN   # padded shard height: all windows in-bounds (142)

_prog_cache: dict = {}


# --------------------------------------------------------------------------- #
# custom DVE ops (registered at import into concourse.dve_ops)                #
# --------------------------------------------------------------------------- #

def _register_ops():
    from concourse import dve_ops
    from concourse.dve_spec import (
        Spec, Src0, Src1, C0, C1, C2, Zero, lower, select, _has_src1,
    )
    from concourse.dve_uop import DveOpSpec

    if "ANT_ROUTE_BIN8" in dve_ops._SUB_OPCODE_FOR_NAME:
        ops = {op.name: op for op in dve_ops.OPS}
        return ops["ANT_ROUTE_BIN8"], ops["ANT_SEL_BIN8"]

    def reg(name, spec, subdim=False):
        row = dve_ops._CUSTOM_DVE_ROW_BASE + len(dve_ops.OPS)
        assert row < 0x20, "custom DVE opcode rows exhausted"
        dve_ops._SUB_OPCODE_FOR_NAME[name] = row
        shas = {}
        for ver in ("v3", "v4"):
            try:
                s = DveOpSpec(name=name, opcode=row,
                              uops=lower(spec, ver=ver),
                              rd1_en=_has_src1(spec))
                shas[ver] = s.sha(ver)
            except Exception:
                pass
        op = dve_ops.DveOp(name, spec, subdim=subdim, uops_sha=shas)
        dve_ops.OPS.append(op)
        dve_ops.CUSTOM_DVE_SPECS[name] = spec
        return op

    def _bc(v):
        return v if isinstance(v, float) else np.asarray(v).reshape(-1, 1)

    # s = x if (x >= lo) & (x < hi) else 0   (lo/hi per-partition scalars)
    route = reg("ANT_ROUTE_BIN8", Spec(
        body=select((Src0 >= C0) & (Src0 < C1), Src0, Zero),
        reference=lambda in0, in1, s0, s1, imm2: np.where(
            (in0 >= _bc(s0)) & (in0 < _bc(s1)), in0, 0.0).astype(np.float32),
    ))

    # masked = (x >= lo) & (x < lo + width) ? g + bias : 0
    #   in0 = g (PSUM), in1 = x (center voxel), s0 = lo, s1 = bias,
    #   imm2 = 1/8 bin width (compile-time literal)
    selb = reg("ANT_SEL_BIN8", Spec(
        body=select((Src1 >= C0) & (Src1 < (C0 + C2)), Src0 + C1, Zero),
        reference=lambda in0, in1, s0, s1, imm2: np.where(
            (in1 >= _bc(s0)) & (in1 < (_bc(s0) + imm2)),
            in0 + _bc(s1), 0.0).astype(np.float32),
    ))
    return route, selb


# --------------------------------------------------------------------------- #
# host-side constant construction                                             #
# --------------------------------------------------------------------------- #

def _band_lhsT(dw_kernel, co_matrix, dwi, dci):
    """lhsT[(q,hs),(p,hs')] = K[dh+1, dwi+1, dci+1, p] * co[p,q], dh=hs-hs',
    for hs' in [1,15), |dh| <= 1."""
    K = np.asarray(dw_kernel, np.float32)       # [3,3,3,1,8]
    co = np.asarray(co_matrix, np.float32)      # [8,8]
    lhsT = np.zeros((128, 128), np.float32)
    hsp = np.arange(1, 15)
    for q in range(NB):
        for p in range(NB):
            for dh in (-1, 0, 1):
                a = K[dh + 1, dwi + 1, dci + 1, 0, p] * co[p, q]
                lhsT[q * WIN + hsp + dh, p * WIN + hsp] = a
    return lhsT


def _make_consts(co_matrix, dw_kernel, dw_bias):
    conv_w = np.stack([_band_lhsT(dw_kernel, co_matrix, dwi, dci)
                       for (dwi, dci) in TAPS])          # [9,128,128]
    # red_w[hw][p*16+hs, h] = 1 iff h == 14*hw + hs - 1, hs in [1,15)
    red_w = np.zeros((NWIN, 128, 128), np.float32)
    for hw in range(NWIN):
        for p in range(NB):
            for hs in range(1, 15):
                h = 14 * hw + hs - 1
                if 0 <= h < H:
                    red_w[hw, p * WIN + hs, h] = 1.0
    part = np.arange(128)
    bins_lo = ((part // WIN) / NB).astype(np.float32).reshape(128, 1)
    bias_p = np.asarray(dw_bias, np.float32)[part // WIN].reshape(128, 1)
    return {
        "conv_w": conv_w.astype(ml_dtypes.bfloat16),
        "red_w": red_w.astype(ml_dtypes.bfloat16),
        "bins_lo": bins_lo,
        "bias_p": bias_p,
    }


def _shard(x, core):
    """Per-core input: [PH, WH, C] with zero h-halo rows and w-halo cols."""
    b, wh = core // 2, core % 2
    xp = np.zeros((PH, WH, C), np.float32)
    lo, hi = wh * WS - 1, wh * WS + WS + 1
    slo, shi = max(0, lo), min(W, hi)
    xp[1:H + 1, slo - lo:shi - lo, :] = x[b, :, slo:shi, :]
    return xp


# --------------------------------------------------------------------------- #
# device program                                                              #
# --------------------------------------------------------------------------- #

def _build_program(reps=1):
    import concourse.mybir as mybir
    import concourse.tile as tile
    from concourse import bacc
    import bass_rust

    def raw_ap(base_ap, dims, offset):
        a = base_ap.copy()
        a.ap = bass_rust.VecI64Pair(dims)
        a.offset = offset
        return a

    ROUTE, SELB = _register_ops()
    f32 = mybir.dt.float32
    bf16 = mybir.dt.bfloat16

    nc = bacc.Bacc("TRN2", target_bir_lowering=False, debug=False)
    x_d = nc.dram_tensor("x_s", [PH, WH, C], f32, kind="ExternalInput")
    cw_d = nc.dram_tensor("conv_w", [9, 128, 128], bf16, kind="ExternalInput")
    rw_d = nc.dram_tensor("red_w", [NWIN, 128, 128], bf16,
                          kind="ExternalInput")
    lo_d = nc.dram_tensor("bins_lo", [128, 1], f32, kind="ExternalInput")
    bi_d = nc.dram_tensor("bias_p", [128, 1], f32, kind="ExternalInput")
    out_d = nc.dram_tensor("out_s", [H, WS, C], f32, kind="ExternalOutput")

    with tile.TileContext(nc) as tc:
        with (
            tc.tile_pool(name="const", bufs=1) as cpool,
            tc.tile_pool(name="xr", bufs=2) as xrpool,
            tc.tile_pool(name="sl", bufs=2) as slpool,
            tc.tile_pool(name="mk", bufs=2) as mkpool,
            tc.tile_pool(name="ost", bufs=2) as ostpool,
            tc.tile_pool(name="ps", bufs=3, space="PSUM") as pspool,
            tc.tile_pool(name="ps2", bufs=2, space="PSUM") as ps2pool,
        ):
            cw = cpool.tile([128, 9 * 128], bf16)
            nc.sync.dma_start(
                cw[:, :],
                raw_ap(cw_d[0], [[128, 128], [128 * 128, 9], [1, 128]], 0))
            rw = cpool.tile([128, NWIN * 128], bf16)
            nc.sync.dma_start(
                rw[:, :],
                raw_ap(rw_d[0], [[128, 128], [128 * 128, NWIN], [1, 128]], 0))
            lo = cpool.tile([128, 1], f32)
            nc.sync.dma_start(lo[:], lo_d[:])
            bi = cpool.tile([128, 1], f32)
            nc.sync.dma_start(bi[:], bi_d[:])
            hi = cpool.tile([128, 1], f32)
            nc.vector.tensor_scalar_add(hi[:], lo[:], 1.0 / NB)

            for rep in range(reps):
              for st in range(NSTRIPES):
                  wb = st * WSTR
                  # ---- load x replicated into (q, hs) per window ----------- #
                  # padded row r = h+1: window hw needs h = 14*hw-1 .. 14*hw+14
                  # -> padded rows 14*hw .. 14*hw+15, all in-bounds.
                  xrm = xrpool.tile([128, NWIN, FSTR_IN], f32, tag="xrm")
                  for hw in range(NWIN):
                      nc.sync.dma_start(
                          xrm[:, hw, :],
                          raw_ap(x_d[0:WIN, 0:WSTR_IN, :],
                                 [[0, 8], [PP, WIN], [1, FSTR_IN]],
                                 14 * hw * PP + wb * C))

                  def xwin(hw):
                      return xrm[:, hw, :]

                  # ---- route to bf16 bin fields (c padded to 66) ------------ #
                  sl = slpool.tile([128, NWIN, WSTR_IN, CP], bf16, tag="sl")
                  # zero the c-border columns once per stripe
                  nc.gpsimd.memset(sl[:, :, :, 0], 0.0)
                  nc.gpsimd.memset(sl[:, :, :, CP - 1], 0.0)
                  for hw in range(NWIN):
                      nc.vector._custom_dve(
                          ROUTE, out=sl[:, hw, :, 1:1 + C],
                          in0=xwin(hw).rearrange("p (w c) -> p w c", c=C),
                          s0=lo[:], s1=hi[:])

                  # ---- conv + select + stripe-accumulated reduce ----------- #
                  p2 = ps2pool.tile([128, 1024], f32, tag="p2",
                                    name=f"p2_{st}")
                  pend = []

                  def flush_tail(pend=pend, st=st, p2=p2):
                      if not pend:
                          return
                      ps, hw, wc = pend.pop(0)
                      mk = mkpool.tile([128, 512], bf16, tag="mk",
                                       name=f"mk_{st}_{hw}_{wc}")
                      xcen = xwin(hw)[:, (wc * 8 + 1) * C:(wc * 8 + 9) * C]
                      nc.vector._custom_dve(
                          SELB, out=mk[:], in0=ps[:],
                          in1=xcen, s0=lo[:], s1=bi[:], imm2=1.0 / NB)
                      nc.tensor.matmul(
                          p2[:, wc * 512:(wc + 1) * 512],
                          rw[:, hw * 128:(hw + 1) * 128], mk[:],
                          start=(hw == 0), stop=(hw == NWIN - 1))

                  for hw in range(NWIN):
                      for wc in range(WSTR // 8):
                          ps = pspool.tile([128, 512], f32, tag="ps",
                                           name=f"ps_{st}_{hw}_{wc}")
                          for ti, (dwi, dci) in enumerate(TAPS):
                              w0 = wc * 8 + 1 + dwi
                              rhs = sl[:, hw, w0:w0 + 8, 1 + dci:1 + dci + C]
                              nc.tensor.matmul(
                                  ps[:], cw[:, ti * 128:(ti + 1) * 128], rhs,
                                  start=(ti == 0), stop=(ti == len(TAPS) - 1))
                          pend.append((ps, hw, wc))
                          if len(pend) > 1:
                              flush_tail()
                  while pend:
                      flush_tail()

                  # ---- drain stripe + store ------------------------------- #
                  ost = ostpool.tile([128, 1024], f32, tag="ost",
                                     name=f"ost_{st}")
                  nc.scalar.copy(ost[:, :], p2[:, :])
                  nc.sync.dma_start(
                      raw_ap(out_d[0:H, 0:WSTR, :],
                             [[WS * C, 128], [1, 1024]], wb * C),
                      ost[:, :])

    nc.compile()
    return nc


def _get_program(reps=1):
    key = reps
    if key not in _prog_cache:
        _prog_cache[key] = _build_program(reps)
    return _prog_cache[key]


# --------------------------------------------------------------------------- #
# entry point                                                                 #
# --------------------------------------------------------------------------- #

def kernel(x, co_matrix, dw_kernel, dw_bias):
    from concourse.bass_utils import run_bass_kernel_spmd

    x = np.asarray(x, np.float32)
    consts = _make_consts(co_matrix, dw_kernel, dw_bias)
    nc = _get_program()

    in_maps = []
    for core in range(NCORES):
        m = {"x_s": _shard(x, core)}
        m.update(consts)
        in_maps.append(m)

    res = run_bass_kernel_spmd(nc, in_maps, core_ids=list(range(NCORES)))
    out = np.zeros((B, H, W, C), np.float32)
    for core in range(NCORES):
        b, wh = core // 2, core % 2
        out[b, :, wh * WS:(wh + 1) * WS, :] = res.results[core]["out_s"]
    return out


# revision 3
# speedup vs baseline: 3031.9072x; 1.0106x over previous
"""Trainium2 Bass kernel for nn_CoLL_78065325572576 (moe_routing).

Reference computation (per voxel v of x[B,H,W,C], nb=8 bins):
    b_v   = floor(8*x_v)                       (bin index)
    temp  = co[i, b_v] * x_v                   (8 channels)
    conv  = depthwise 3x3x3 conv over (H,W,C)  (SAME pad, 8 channels)
    out_v = conv[v, b_v] + bias[b_v]

Kernel formulation (all equalities exact):
    s_q[v]  = x_v * 1[b_v == q]                 (mask-routed fields)
    out_v   = sum_p 1[b_v==p] * ( sum_{dv,q} K[dv,p]*co[p,q] * s_q[v+dv] + bias[p] )

Device mapping (per core, pure data-parallel over 8 cores = batch x W-half):
  - per h-window (16 rows, stride 14): ONE partition-replicating DMA loads
    x rows into all 8 bin-groups (src AP [[0,8],[row,16],[1,F]]).
  - ROUTE (custom DVE op): s = x masked per bin-group, bf16.
  - CONV: 9 accumulating banded bf16 matmuls on TensorE; lhsT
    [(q,hs),(p,hs')] = K[hs-hs'+1, dw+1, dc+1, p]*co[p,q] folds the
    8x8 channel mix and the dh taps; (dw,dc) are free-dim AP shifts.
  - SELECT (custom DVE op): masked = (x in bin p) ? g + bias[p] : 0 (bf16).
  - REDUCE: per-window banded bf16 matmul scatters the 14 valid rows into a
    stripe-wide PSUM accumulator [128, 1024] (10 windows accumulate).
  - One ScalarE drain + one output DMA per stripe.
"""

import numpy as np
import ml_dtypes

NB = 8
B, H, W, C = 4, 128, 128, 64
WS = 64            # output W per core
WH = WS + 2        # input W incl. halo
WIN = 16           # h-window rows (one partition group)
VALID = 14         # valid output rows per window
NWIN = 10          # h windows (stride 14): covers h in [0,128)
NCORES = 8
NSTRIPES = 4
WSTR = WS // NSTRIPES        # 16 output w per stripe
WSTR_IN = WSTR + 2           # 18 input w per stripe
FSTR_IN = WSTR_IN * C        # 1152
CP = C + 2                   # c padded with zero border cols in routed tensor
TAPS = [(dw, dc) for dc in (0, -1, 1) for dw in (-1, 0, 1)]  # dc=0 first
PP = WH * C                  # x_s row pitch (elements)
PH = 14 * (NWIN - 1) + WIN   # padded shard height: all windows in-bounds (142)

_prog_cache: dict = {}


# --------------------------------------------------------------------------- #
# custom DVE ops (registered at import into concourse.dve_ops)                #
# --------------------------------------------------------------------------- #

def _register_ops():
    from concourse import dve_ops
    from concourse.dve_spec import (
        Spec, Src0, Src1, C0, C1, C2, Zero, lower, select, _has_src1,
    )
    from concourse.dve_uop import DveOpSpec

    if "ANT_ROUTE_BIN8" in dve_ops._SUB_OPCODE_FOR_NAME:
        ops = {op.name: op for op in dve_ops.OPS}
        return ops["ANT_ROUTE_BIN8"], ops["ANT_SEL_BIN8"]

    def reg(name, spec, subdim=False):
        row = dve_ops._CUSTOM_DVE_ROW_BASE + len(dve_ops.OPS)
        assert row < 0x20, "custom DVE opcode rows exhausted"
        dve_ops._SUB_OPCODE_FOR_NAME[name] = row
        shas = {}
        for ver in ("v3", "v4"):
            try:
                s = DveOpSpec(name=name, opcode=row,
                              uops=lower(spec, ver=ver),
                              rd1_en=_has_src1(spec))
                shas[ver] = s.sha(ver)
            except Exception:
                pass
        op = dve_ops.DveOp(name, spec, subdim=subdim, uops_sha=shas)
        dve_ops.OPS.append(op)
        dve_ops.CUSTOM_DVE_SPECS[name] = spec
        return op

    def _bc(v):
        return v if isinstance(v, float) else np.asarray(v).reshape(-1, 1)

    # s = x if (x >= lo) & (x < hi) else 0   (lo/hi per-partition scalars)
    route = reg("ANT_ROUTE_BIN8", Spec(
        body=select((Src0 >= C0) & (Src0 < C1), Src0, Zero),
        reference=lambda in0, in1, s0, s1, imm2: np.where(
            (in0 >= _bc(s0)) & (in0 < _bc(s1)), in0, 0.0).astype(np.float32),
    ))

    # masked = (x >= lo) & (x < lo + width) ? g + bias : 0
    #   in0 = g (PSUM), in1 = x (center voxel), s0 = lo, s1 = bias,
    #   imm2 = 1/8 bin width (compile-time literal)
    selb = reg("ANT_SEL_BIN8", Spec(
        body=select((Src1 >= C0) & (Src1 < (C0 + C2)), Src0 + C1, Zero),
        reference=lambda in0, in1, s0, s1, imm2: np.where(
            (in1 >= _bc(s0)) & (in1 < (_bc(s0) + imm2)),
            in0 + _bc(s1), 0.0).astype(np.float32),
    ))
    return route, selb


# --------------------------------------------------------------------------- #
# host-side constant construction                                             #
# --------------------------------------------------------------------------- #

def _band_lhsT(dw_kernel, co_matrix, dwi, dci):
    """lhsT[(q,hs),(p,hs')] = K[dh+1, dwi+1, dci+1, p] * co[p,q], dh=hs-hs',
    for hs' in [1,15), |dh| <= 1."""
    K = np.asarray(dw_kernel, np.float32)       # [3,3,3,1,8]
    co = np.asarray(co_matrix, np.float32)      # [8,8]
    lhsT = np.zeros((128, 128), np.float32)
    hsp = np.arange(1, 15)
    for q in range(NB):
        for p in range(NB):
            for dh in (-1, 0, 1):
                a = K[dh + 1, dwi + 1, dci + 1, 0, p] * co[p, q]
                lhsT[q * WIN + hsp + dh, p * WIN + hsp] = a
    return lhsT


def _make_consts(co_matrix, dw_kernel, dw_bias):
    conv_w = np.stack([_band_lhsT(dw_kernel, co_matrix, dwi, dci)
                       for (dwi, dci) in TAPS])          # [9,128,128]
    # red_w[hw][p*16+hs, h] = 1 iff h == 14*hw + hs - 1, hs in [1,15)
    red_w = np.zeros((NWIN, 128, 128), np.float32)
    for hw in range(NWIN):
        for p in range(NB):
            for hs in range(1, 15):
                h = 14 * hw + hs - 1
                if 0 <= h < H:
                    red_w[hw, p * WIN + hs, h] = 1.0
    part = np.arange(128)
    bins_lo = ((part // WIN) / NB).astype(np.float32).reshape(128, 1)
    bias_p = np.asarray(dw_bias, np.float32)[part // WIN].reshape(128, 1)
    return {
        "conv_w": conv_w.astype(ml_dtypes.bfloat16),
        "red_w": red_w.astype(ml_dtypes.bfloat16),
        "bins_lo": bins_lo,
        "bias_p": bias_p,
    }


def _shard(x, core):
    """Per-core input: [PH, WH, C] with zero h-halo rows and w-halo cols."""
    b, wh = core // 2, core % 2
    xp = np.zeros((PH, WH, C), np.float32)
    lo, hi = wh * WS - 1, wh * WS + WS + 1
    slo, shi = max(0, lo), min(W, hi)
    xp[1:H + 1, slo - lo:shi - lo, :] = x[b, :, slo:shi, :]
    return xp


# --------------------------------------------------------------------------- #
# device program                                                              #
# --------------------------------------------------------------------------- #

def _build_program(reps=1):
    import concourse.mybir as mybir
    import concourse.tile as tile
    from concourse import bacc
    import bass_rust

    def raw_ap(base_ap, dims, offset):
        a = base_ap.copy()
        a.ap = bass_rust.VecI64Pair(dims)
        a.offset = offset
        return a

    ROUTE, SELB = _register_ops()
    f32 = mybir.dt.float32
    bf16 = mybir.dt.bfloat16

    nc = bacc.Bacc("TRN2", target_bir_lowering=False, debug=False)
    x_d = nc.dram_tensor("x_s", [PH, WH, C], f32, kind="ExternalInput")
    cw_d = nc.dram_tensor("conv_w", [9, 128, 128], bf16, kind="ExternalInput")
    rw_d = nc.dram_tensor("red_w", [NWIN, 128, 128], bf16,
                          kind="ExternalInput")
    lo_d = nc.dram_tensor("bins_lo", [128, 1], f32, kind="ExternalInput")
    bi_d = nc.dram_tensor("bias_p", [128, 1], f32, kind="ExternalInput")
    out_d = nc.dram_tensor("out_s", [H, WS, C], f32, kind="ExternalOutput")

    with tile.TileContext(nc) as tc:
        with (
            tc.tile_pool(name="const", bufs=1) as cpool,
            tc.tile_pool(name="xr", bufs=2) as xrpool,
            tc.tile_pool(name="sl", bufs=2) as slpool,
            tc.tile_pool(name="mk", bufs=2) as mkpool,
            tc.tile_pool(name="ost", bufs=2) as ostpool,
            tc.tile_pool(name="ps", bufs=3, space="PSUM") as pspool,
            tc.tile_pool(name="ps2", bufs=2, space="PSUM") as ps2pool,
        ):
            # startup critical chain: lo/hi gate the first ROUTE, cw gates the
            # first conv matmul; load those before the bulkier rw (only
            # needed once the first chunk's SELB completes).
            lo = cpool.tile([128, 1], f32)
            nc.sync.dma_start(lo[:], lo_d[:])
            bi = cpool.tile([128, 1], f32)
            nc.sync.dma_start(bi[:], bi_d[:])
            hi = cpool.tile([128, 1], f32)
            nc.vector.tensor_scalar_add(hi[:], lo[:], 1.0 / NB)
            cw = cpool.tile([128, 9 * 128], bf16)
            nc.sync.dma_start(
                cw[:, :],
                raw_ap(cw_d[0], [[128, 128], [128 * 128, 9], [1, 128]], 0))
            rw = cpool.tile([128, NWIN * 128], bf16)
            nc.gpsimd.dma_start(
                rw[:, :],
                raw_ap(rw_d[0], [[128, 128], [128 * 128, NWIN], [1, 128]], 0))

            for rep in range(reps):
              for st in range(NSTRIPES):
                  wb = st * WSTR
                  # ---- load x replicated into (q, hs) per window ----------- #
                  # padded row r = h+1: window hw needs h = 14*hw-1 .. 14*hw+14
                  # -> padded rows 14*hw .. 14*hw+15, all in-bounds.
                  xrm = xrpool.tile([128, NWIN, FSTR_IN], f32, tag="xrm")
                  for hw in range(NWIN):
                      nc.sync.dma_start(
                          xrm[:, hw, :],
                          raw_ap(x_d[0:WIN, 0:WSTR_IN, :],
                                 [[0, 8], [PP, WIN], [1, FSTR_IN]],
                                 14 * hw * PP + wb * C))

                  def xwin(hw):
                      return xrm[:, hw, :]

                  # ---- route to bf16 bin fields (c padded to 66) ------------ #
                  sl = slpool.tile([128, NWIN, WSTR_IN, CP], bf16, tag="sl")
                  # zero the c-border columns once per stripe
                  nc.gpsimd.memset(sl[:, :, :, 0], 0.0)
                  nc.gpsimd.memset(sl[:, :, :, CP - 1], 0.0)
                  for hw in range(NWIN):
                      nc.vector._custom_dve(
                          ROUTE, out=sl[:, hw, :, 1:1 + C],
                          in0=xwin(hw).rearrange("p (w c) -> p w c", c=C),
                          s0=lo[:], s1=hi[:])

                  # ---- conv + select + stripe-accumulated reduce ----------- #
                  p2 = ps2pool.tile([128, 1024], f32, tag="p2",
                                    name=f"p2_{st}")
                  pend = []

                  def flush_tail(pend=pend, st=st, p2=p2):
                      if not pend:
                          return
                      ps, hw, wc = pend.pop(0)
                      mk = mkpool.tile([128, 512], bf16, tag="mk",
                                       name=f"mk_{st}_{hw}_{wc}")
                      xcen = xwin(hw)[:, (wc * 8 + 1) * C:(wc * 8 + 9) * C]
                      nc.vector._custom_dve(
                          SELB, out=mk[:], in0=ps[:],
                          in1=xcen, s0=lo[:], s1=bi[:], imm2=1.0 / NB)
                      nc.tensor.matmul(
                          p2[:, wc * 512:(wc + 1) * 512],
                          rw[:, hw * 128:(hw + 1) * 128], mk[:],
                          start=(hw == 0), stop=(hw == NWIN - 1))

                  for hw in range(NWIN):
                      for wc in range(WSTR // 8):
                          ps = pspool.tile([128, 512], f32, tag="ps",
                                           name=f"ps_{st}_{hw}_{wc}")
                          for ti, (dwi, dci) in enumerate(TAPS):
                              w0 = wc * 8 + 1 + dwi
                              rhs = sl[:, hw, w0:w0 + 8, 1 + dci:1 + dci + C]
                              nc.tensor.matmul(
                                  ps[:], cw[:, ti * 128:(ti + 1) * 128], rhs,
                                  start=(ti == 0), stop=(ti == len(TAPS) - 1))
                          pend.append((ps, hw, wc))
                          if len(pend) > 1:
                              flush_tail()
                  while pend:
                      flush_tail()

                  # ---- drain stripe + store ------------------------------- #
                  ost = ostpool.tile([128, 1024], f32, tag="ost",
                                     name=f"ost_{st}")
                  nc.scalar.copy(ost[:, :], p2[:, :])
                  nc.sync.dma_start(
                      raw_ap(out_d[0:H, 0:WSTR, :],
                             [[WS * C, 128], [1, 1024]], wb * C),
                      ost[:, :])

    nc.compile()
    return nc


def _get_program(reps=1):
    key = reps
    if key not in _prog_cache:
        _prog_cache[key] = _build_program(reps)
    return _prog_cache[key]


# --------------------------------------------------------------------------- #
# entry point                                                                 #
# --------------------------------------------------------------------------- #

def kernel(x, co_matrix, dw_kernel, dw_bias):
    from concourse.bass_utils import run_bass_kernel_spmd

    x = np.asarray(x, np.float32)
    consts = _make_consts(co_matrix, dw_kernel, dw_bias)
    nc = _get_program()

    in_maps = []
    for core in range(NCORES):
        m = {"x_s": _shard(x, core)}
        m.update(consts)
        in_maps.append(m)

    res = run_bass_kernel_spmd(nc, in_maps, core_ids=list(range(NCORES)))
    out = np.zeros((B, H, W, C), np.float32)
    for core in range(NCORES):
        b, wh = core // 2, core % 2
        out[b, :, wh * WS:(wh + 1) * WS, :] = res.results[core]["out_s"]
    return out
